# revision 13
# baseline (speedup 1.0000x reference)
"""CARAFE content-aware upsampling (scale=2, K=5, encoder 3x3) on 8 TRN2 NeuronCores.

Sharding: 8 shards = batch(4) x H-halves(2), pure data parallel (1-row halo per
shard handled host-side). Channel-major fp16 pipeline, v2:

  1. compress 1x1 conv       : PE matmul, ACT drain fused with +cb bias
  2. encoder 3x3 conv        : 9 accumulating PE matmuls on padded k1
  3. e = exp(enc + eb)       : ACT
  4. S, lnS, r_exp           : PE column-sum matmul, ACT Ln, PE 4->100 expand,
                               ACT Exp(-x) drain  -> r_exp = 1/S per channel row
  5. e_norm = e * r_exp      : one DVE TT mul [100, PIX]; masks are now
                               pre-normalized at the e level
  6. mask supply (36 taps)   :  - dx in {-1,0} (24 taps): Mu rows via A-matmul,
                                  ACT drain, DRAM bounce, per-(q,half) stride-0
                                  broadcast DMA into [128, 6, 1024] tiles
                               - dx = +1 (12 taps): PE broadcast matmul
                                  (stationary = A column replicated 128x) from
                                  e_norm, drained by ACT (q0-2) / DVE (q3)
  7. products                : batched DVE TT muls (6-tap and 3-tap batches,
                               fp16 2x mode) against dx-stacked x copies
  8. accumulate              : 9-term PE identity matmuls into PSUM (fp32)
  9. out                     : ACT/DVE cast drain to fp16, contiguous per-subgrid
                               DRAM store; host does pixel-shuffle + fp32 cast
"""

import numpy as np

SCALE, KK, EK = 2, 5, 3
B, C, H, W = 4, 128, 64, 64
CC, KC = 64, 100
HS = H // 2          # 32 interior rows per shard
PIX = HS * W
HALF = PIX // 2      # 1024
NCORES = 8

_PROGRAM = None


def _build_A():
    A = np.zeros((KC, 40), dtype=np.float32)
    for r1 in range(2):
        for r2 in range(2):
            q = 2 * r1 + r2
            for i in range(KK):
                for j in range(KK):
                    dy = (r1 + i - 2) // 2
                    dx = (r2 + j - 2) // 2
                    tidx = (dy + 1) * 3 + (dx + 1)
                    A[4 * (5 * i + j) + q, q * 9 + tidx] += 1.0
            A[np.arange(q, KC, 4), 36 + q] = 1.0
    return A


def _patch_act_tables():
    # Force every activation (Exp/Ln/Identity/Copy) onto the one table set
    # that contains them all, so the kernel pays a single ACT_TABLE_LOAD
    # instead of thrashing between exp_and_others and natural_log sets.
    from concourse import bacc, hw_specs
    if getattr(bacc, "_carafe_act_patch", False):
        return
    orig = hw_specs.get_activation_tables

    def patched(arch):
        t = orig(arch)
        return {
            k: (v if k == "natural_log_exp_and_others" else set())
            for k, v in t.items()
        }

    hw_specs.get_activation_tables = patched
    bacc.get_activation_tables = patched
    bacc._carafe_act_patch = True


def _build_program():
    import concourse.bass as bass
    import concourse.tile as tile
    from concourse.tile import add_dep_helper
    from concourse import bacc, mybir

    _patch_act_tables()

    f32 = mybir.dt.float32
    f16 = mybir.dt.float16
    AF = mybir.ActivationFunctionType

    nc = bacc.Bacc("TRN2", target_bir_lowering=False, debug=False,
                   num_devices=NCORES)

    xin = nc.declare_dram_parameter("xs", [C, HS + 2, W], f32, isOutput=False)
    cwT = nc.declare_dram_parameter("comp_wT", [C, CC], f16, isOutput=False)
    cb = nc.declare_dram_parameter("comp_b", [CC, 1], f32, isOutput=False)
    ewT = nc.declare_dram_parameter("enc_wT", [CC, 9, KC], f16, isOutput=False)
    eb = nc.declare_dram_parameter("enc_b", [KC, 1], f32, isOutput=False)
    out = nc.declare_dram_parameter("out", [C, 4, HS, W], f16, isOutput=True)

    A_np = _build_A().astype(np.float16)
    A_dram = nc.inline_tensor(A_np, name="A_cmb")
    # PE-broadcast stationaries: for each (q, dy) the dx=+1 mask column of A,
    # replicated across 128 output partitions.
    A1 = np.zeros((KC, 12, 128), dtype=np.float16)
    for q in range(4):
        for k in range(3):          # k = dy + 1
            tidx = 3 * k + 2        # dx = +1
            A1[:, 3 * q + k, :] = A_np[:, 9 * q + tidx][:, None]
    A1_dram = nc.inline_tensor(A1.reshape(KC, 12 * 128), name="A1_bcast")
    R100 = np.zeros((4, KC), dtype=np.float16)
    for j in range(KC):
        R100[j % 4, j] = 1.0
    R100_dram = nc.inline_tensor(R100, name="R100")
    I_dram = nc.inline_tensor(np.eye(128, dtype=np.float16), name="ident")

    mu_dram = nc.dram_tensor("mu_bounce", [2, 36, HALF], f16)

    with tile.TileContext(nc) as tc:
        with (
            tc.tile_pool(name="singles", bufs=1) as singles,
            tc.tile_pool(name="mc", bufs=3) as mc,
            tc.tile_pool(name="pd", bufs=2) as pd,
            tc.tile_pool(name="pr", bufs=2) as pr,
        ):
            # persistent SBUF
            x16 = singles.tile([C, 3, HS + 2, W], f16, tag="x16")  # dx -1,0,+1
            k1pad = singles.tile([CC, HS + 2, W + 2], f16, tag="k1pad")
            e_sb = singles.tile([KC, HS, W], f16, tag="e_sb")
            en_sb = singles.tile([KC, HS, W], f16, tag="en_sb")
            rexp16 = singles.tile([KC, HS, W], f16, tag="rexp16")
            lnS = singles.tile([4, HS, W], f16, tag="lnS")
            mu16 = singles.tile([36, HS, W], f16, tag="mu16")
            out16 = singles.tile([C, 4, HS, W], f16, tag="out16")
            cwT_sb = singles.tile([C, CC], f16, tag="cwT")
            cb_sb = singles.tile([CC, 1], f32, tag="cb")
            ewT_sb = singles.tile([CC, 9, KC], f16, tag="ewT")
            eb_sb = singles.tile([KC, 1], f32, tag="eb")
            A_sb = singles.tile([KC, 40], f16, tag="A_sb")
            A1_sb = singles.tile([KC, 12 * 128], f16, tag="A1_sb")
            R100_sb = singles.tile([4, KC], f16, tag="R100_sb")
            id_sb = singles.tile([128, 128], f16, tag="id_sb")

            nc.gpsimd.memset(x16[:, 0, :, 0:1], 0.0)
            nc.gpsimd.memset(x16[:, 2, :, W - 1 : W], 0.0)
            nc.gpsimd.memset(k1pad[:, :, 0:1], 0.0)
            nc.gpsimd.memset(k1pad[:, :, W + 1 : W + 2], 0.0)

            # x load with fp32 -> fp16 cast (SWDGE); dx-shifted copies on GPSIMD
            nc.gpsimd.dma_start(out=x16[:, 1, 0:17, :], in_=xin[:, 0:17, :])
            nc.gpsimd.dma_start(out=x16[:, 1, 17:, :], in_=xin[:, 17:, :])
            nc.sync.dma_start(out=cwT_sb, in_=cwT[:])
            nc.sync.dma_start(out=cb_sb, in_=cb[:])
            nc.sync.dma_start(out=ewT_sb, in_=ewT[:])
            nc.sync.dma_start(out=eb_sb, in_=eb[:])
            nc.sync.dma_start(out=A_sb, in_=A_dram[:])
            nc.sync.dma_start(out=A1_sb, in_=A1_dram[:])
            nc.sync.dma_start(out=R100_sb, in_=R100_dram[:])
            nc.sync.dma_start(out=id_sb, in_=I_dram[:])

            nc.vector.tensor_copy(x16[:, 0, :, 1:W], x16[:, 1, :, 0 : W - 1])
            nc.vector.tensor_copy(x16[:, 2, :, 0 : W - 1], x16[:, 1, :, 1:W])

            bounce = [None, None]
            with (
                tc.tile_pool(name="ps_m", bufs=2, space="PSUM") as ps_m,
                tc.tile_pool(name="ps_n", bufs=1, space="PSUM") as ps_n,
            ):
                # compress conv over all 34 rows, 8-row chunks
                for ck, (r0, r1) in enumerate(
                        [(0, 8), (8, 16), (16, 24), (24, 32), (32, 34)]):
                    ps_k = ps_m.tile([CC, r1 - r0, W], f32, tag="ps_k",
                                     name=f"psk_{ck}")
                    nc.tensor.matmul(ps_k, cwT_sb, x16[:, 1, r0:r1, :],
                                     start=True, stop=True)
                    nc.scalar.activation(k1pad[:, r0:r1, 1 : 1 + W],
                                         ps_k, AF.Identity, bias=cb_sb,
                                         scale=1.0)

                for h in range(2):
                    y0 = 16 * h
                    # encoder 3x3 on padded grid -> exp -> e
                    ps_e = ps_m.tile([KC, 16, W], f32, tag="ps_e",
                                     name=f"pse_{h}")
                    for hh in range(2):
                        h0 = y0 + 8 * hh
                        for di in range(3):
                            for dj in range(3):
                                tap = di * 3 + dj
                                nc.tensor.matmul(
                                    ps_e[:, 8 * hh : 8 * hh + 8, :],
                                    ewT_sb[:, tap, :],
                                    k1pad[:, h0 + di : h0 + di + 8, dj : dj + W],
                                    start=(tap == 0), stop=(tap == 8))
                    nc.scalar.activation(e_sb[:, y0 : y0 + 16, :], ps_e, AF.Exp,
                                         bias=eb_sb, scale=1.0)

                    # softmax denominators -> lnS -> r_exp (1/S per e-row)
                    ps_s = ps_n.tile([4, 16, W], f32, tag="ps_s", name=f"pss_{h}")
                    for hh in range(2):
                        nc.tensor.matmul(ps_s[:, 8 * hh : 8 * hh + 8, :],
                                         A_sb[:, 36:40],
                                         e_sb[:, y0 + 8 * hh : y0 + 8 * hh + 8, :],
                                         start=True, stop=True)
                    nc.scalar.activation(lnS[:, y0 : y0 + 16, :], ps_s, AF.Ln)
                    ps_r = ps_m.tile([KC, 16, W], f32, tag="ps_e",
                                     name=f"psr_{h}")
                    for hh in range(2):
                        nc.tensor.matmul(ps_r[:, 8 * hh : 8 * hh + 8, :],
                                         R100_sb,
                                         lnS[:, y0 + 8 * hh : y0 + 8 * hh + 8, :],
                                         start=True, stop=True)
                    nc.scalar.activation(rexp16[:, y0 : y0 + 16, :], ps_r,
                                         AF.Exp, scale=-1.0)
                    # normalized e
                    nc.vector.tensor_mul(en_sb[:, y0 : y0 + 16, :],
                                         e_sb[:, y0 : y0 + 16, :],
                                         rexp16[:, y0 : y0 + 16, :])

                    # normalized masks for the DMA-broadcast taps
                    ps_mu = ps_n.tile([36, 16, W], f32, tag="ps_s",
                                      name=f"psmu_{h}")
                    for hh in range(2):
                        nc.tensor.matmul(ps_mu[:, 8 * hh : 8 * hh + 8, :],
                                         A_sb[:, 0:36],
                                         en_sb[:, y0 + 8 * hh : y0 + 8 * hh + 8, :],
                                         start=True, stop=True)
                    nc.scalar.activation(mu16[:, y0 : y0 + 16, :], ps_mu, AF.Copy)
                    bounce[h] = nc.sync.dma_start(
                        out=mu_dram[h], in_=mu16[:, y0 : y0 + 16, :])

            # reassembly main loop (h-major: all of half 0 runs while the
            # half-1 mask phase is still in flight)
            with (
                tc.tile_pool(name="ps_acc", bufs=2, space="PSUM") as ps_acc,
                tc.tile_pool(name="ps_b", bufs=2, space="PSUM") as ps_b,
            ):
                for h in range(2):
                    for q in range(4):
                        y0 = 16 * h
                        # 6-tap broadcast (dx in {-1, 0}) from DRAM bounce
                        mcast = mc.tile([C, 6, HALF], f16, tag="mcast")
                        mflat = mcast.rearrange("p t f -> p (t f)")
                        src = bass.AP(
                            tensor=mu_dram,
                            offset=(h * 36 + q * 9) * HALF,
                            ap=[[0, C], [3 * HALF, 3], [HALF, 2], [1, HALF]])
                        bc = nc.sync.dma_start(out=mflat, in_=src)
                        add_dep_helper(bc.ins, bounce[h].ins, sync=True,
                                       reason="mask broadcast after bounce")

                        # 3-tap PE broadcast (dx = +1) from e_norm
                        pdrain = pd.tile([C, 3, HALF], f16, tag="pdrain")
                        for k in range(3):
                            ps_bc = ps_b.tile([C, HALF], f32, tag="ps_bc",
                                              name=f"psbc_{q}_{h}_{k}")
                            a1 = A1_sb[:, (3 * q + k) * 128 : (3 * q + k + 1) * 128]
                            for hh in range(2):
                                nc.tensor.matmul(
                                    ps_bc[:, 512 * hh : 512 * hh + 512],
                                    a1,
                                    en_sb[:, y0 + 8 * hh : y0 + 8 * hh + 8, :],
                                    start=True, stop=True)
                            if q < 3:
                                nc.scalar.activation(pdrain[:, k], ps_bc, AF.Copy)
                            else:
                                nc.vector.tensor_copy(pdrain[:, k], ps_bc)

                        # products: batched fp16 TT muls
                        prod = pr.tile([C, 9, HALF], f16, tag="prod")
                        pp = 3 * (HS + 2) * W  # x16 partition pitch (elements)
                        xa = bass.AP(
                            tensor=x16.tensor, offset=x16.offset + 16 * h * W,
                            ap=[[pp, C], [W, 3], [(HS + 2) * W, 2], [1, HALF]])
                        nc.vector.tensor_mul(
                            prod[:, 0:6].rearrange("p (a b) f -> p a b f", a=3),
                            xa, mcast.rearrange("p (a b) f -> p a b f", a=3))
                        xb = bass.AP(
                            tensor=x16.tensor,
                            offset=x16.offset + 2 * (HS + 2) * W + 16 * h * W,
                            ap=[[pp, C], [W, 3], [1, HALF]])
                        if q < 3:
                            nc.vector.tensor_mul(prod[:, 6:9], xb, pdrain)
                        else:
                            nc.gpsimd.tensor_mul(prod[:, 6:9], xb, pdrain)

                        # 9-term accumulate on PE
                        acc = ps_acc.tile([C, HALF], f32, tag="acc",
                                          name=f"acc_{q}_{h}")
                        for t in range(9):
                            for c2 in range(2):
                                nc.tensor.matmul(
                                    acc[:, 512 * c2 : 512 * c2 + 512], id_sb,
                                    prod[:, t, 512 * c2 : 512 * c2 + 512],
                                    start=(t == 0), stop=(t == 8),
                                    skip_group_check=True)
                        if q < 2:
                            nc.scalar.activation(
                                out16[:, q, y0 : y0 + 16, :], acc, AF.Copy)
                        else:
                            nc.vector.tensor_copy(
                                out16[:, q, y0 : y0 + 16, :], acc)
                        nc.sync.dma_start(out=out[:, q, y0 : y0 + 16, :],
                                          in_=out16[:, q, y0 : y0 + 16, :])

    nc.compile()
    return nc


def _get_program():
    global _PROGRAM
    if _PROGRAM is None:
        _PROGRAM = _build_program()
    return _PROGRAM


def _shard_inputs(x, comp_w, comp_b, enc_w, enc_b):
    comp_wT = np.ascontiguousarray(comp_w[:, :, 0, 0].T.astype(np.float16))
    enc_wT = np.ascontiguousarray(
        np.transpose(enc_w.reshape(KC, CC, 9), (1, 2, 0)).astype(np.float16))
    cb = np.ascontiguousarray(comp_b.astype(np.float32).reshape(CC, 1))
    eb = np.ascontiguousarray(enc_b.astype(np.float32).reshape(KC, 1))
    in_maps = []
    for core in range(NCORES):
        b, h = divmod(core, 2)
        xs = np.zeros((C, HS + 2, W), dtype=np.float32)
        lo = h * HS - 1
        s0, s1 = max(0, lo), min(H, lo + HS + 2)
        xs[:, s0 - lo : s1 - lo, :] = x[b, :, s0:s1, :]
        in_maps.append({
            "xs": np.ascontiguousarray(xs),
            "comp_wT": comp_wT,
            "comp_b": cb,
            "enc_wT": enc_wT,
            "enc_b": eb,
        })
    return in_maps


def _run(inputs, trace=False):
    from concourse.bass_utils import run_bass_kernel_spmd

    nc = _get_program()
    in_maps = _shard_inputs(**inputs)
    res = run_bass_kernel_spmd(nc, in_maps, list(range(NCORES)), trace=trace)
    out = np.empty((B, C, 2 * H, 2 * W), dtype=np.float32)
    for core in range(NCORES):
        b, h = divmod(core, 2)
        o = res.results[core]["out"].astype(np.float32)  # [C, 4, HS, W]
        blk = out[b, :, h * 2 * HS : (h + 1) * 2 * HS, :]
        for r1 in range(2):
            for r2 in range(2):
                blk[:, r1::2, r2::2] = o[:, 2 * r1 + r2]
    return out, res.exec_time_ns


def kernel(x, comp_w, comp_b, enc_w, enc_b):
    out, _ = _run(dict(x=np.asarray(x), comp_w=np.asarray(comp_w),
                       comp_b=np.asarray(comp_b), enc_w=np.asarray(enc_w),
                       enc_b=np.asarray(enc_b)))
    return out


# revision 17
# speedup vs baseline: 1.2878x; 1.2878x over previous
"""CARAFE content-aware upsampling (scale=2, K=5, encoder 3x3) on 8 TRN2 NeuronCores.

Sharding: 8 shards = batch(4) x H-halves(2), pure data parallel (1-row halo per
shard handled host-side). Channel-major fp16 pipeline, v2:

  1. compress 1x1 conv       : PE matmul, ACT drain fused with +cb bias
  2. encoder 3x3 conv        : 9 accumulating PE matmuls on padded k1
  3. e = exp(enc + eb)       : ACT
  4. S, lnS, r_exp           : PE column-sum matmul, ACT Ln, PE 4->100 expand,
                               ACT Exp(-x) drain  -> r_exp = 1/S per channel row
  5. e_norm = e * r_exp      : one DVE TT mul [100, PIX]; masks are now
                               pre-normalized at the e level
  6. mask supply (36 taps)   :  - dx in {-1,0} (24 taps): Mu rows via A-matmul,
                                  ACT drain, DRAM bounce, per-(q,half) stride-0
                                  broadcast DMA into [128, 6, 1024] tiles
                               - dx = +1 (12 taps): PE broadcast matmul
                                  (stationary = A column replicated 128x) from
                                  e_norm, drained by ACT (q0-2) / DVE (q3)
  7. products                : batched DVE TT muls (6-tap and 3-tap batches,
                               fp16 2x mode) against dx-stacked x copies
  8. accumulate              : 9-term PE identity matmuls into PSUM (fp32)
  9. out                     : ACT/DVE cast drain to fp16, contiguous per-subgrid
                               DRAM store; host does pixel-shuffle + fp32 cast
"""

import numpy as np

SCALE, KK, EK = 2, 5, 3
B, C, H, W = 4, 128, 64, 64
CC, KC = 64, 100
HS = H // 2          # 32 interior rows per shard
PIX = HS * W
HALF = PIX // 2      # 1024
NCORES = 8

_PROGRAM = None


def _build_A():
    A = np.zeros((KC, 40), dtype=np.float32)
    for r1 in range(2):
        for r2 in range(2):
            q = 2 * r1 + r2
            for i in range(KK):
                for j in range(KK):
                    dy = (r1 + i - 2) // 2
                    dx = (r2 + j - 2) // 2
                    tidx = (dy + 1) * 3 + (dx + 1)
                    A[4 * (5 * i + j) + q, q * 9 + tidx] += 1.0
            A[np.arange(q, KC, 4), 36 + q] = 1.0
    return A


def _patch_act_tables():
    # Force every activation (Exp/Ln/Identity/Copy) onto the one table set
    # that contains them all, so the kernel pays a single ACT_TABLE_LOAD
    # instead of thrashing between exp_and_others and natural_log sets.
    from concourse import bacc, hw_specs
    if getattr(bacc, "_carafe_act_patch", False):
        return
    orig = hw_specs.get_activation_tables

    def patched(arch):
        t = orig(arch)
        return {
            k: (v if k == "natural_log_exp_and_others" else set())
            for k, v in t.items()
        }

    hw_specs.get_activation_tables = patched
    bacc.get_activation_tables = patched
    bacc._carafe_act_patch = True


def _build_program():
    import concourse.bass as bass
    import concourse.tile as tile
    from concourse.tile import add_dep_helper
    from concourse import bacc, mybir

    _patch_act_tables()

    f32 = mybir.dt.float32
    f16 = mybir.dt.float16
    AF = mybir.ActivationFunctionType

    nc = bacc.Bacc("TRN2", target_bir_lowering=False, debug=False,
                   num_devices=NCORES)

    xin = nc.declare_dram_parameter("xs", [C, HS + 2, W], f32, isOutput=False)
    cwT = nc.declare_dram_parameter("comp_wT", [C, CC], f16, isOutput=False)
    cb = nc.declare_dram_parameter("comp_b", [CC, 1], f32, isOutput=False)
    ewT = nc.declare_dram_parameter("enc_wT", [CC, 9, KC], f16, isOutput=False)
    eb = nc.declare_dram_parameter("enc_b", [KC, 1], f32, isOutput=False)
    out = nc.declare_dram_parameter("out", [C, 4, HS, W], f16, isOutput=True)

    A_np = _build_A().astype(np.float16)
    A_dram = nc.inline_tensor(A_np, name="A_cmb")
    # PE-broadcast stationaries: for each (q, dy) the dx=+1 mask column of A,
    # replicated across 128 output partitions.
    A1 = np.zeros((KC, 12, 128), dtype=np.float16)
    for q in range(4):
        for k in range(3):          # k = dy + 1
            tidx = 3 * k + 2        # dx = +1
            A1[:, 3 * q + k, :] = A_np[:, 9 * q + tidx][:, None]
    A1_dram = nc.inline_tensor(A1.reshape(KC, 12 * 128), name="A1_bcast")
    R100 = np.zeros((4, KC), dtype=np.float16)
    for j in range(KC):
        R100[j % 4, j] = 1.0
    R100_dram = nc.inline_tensor(R100, name="R100")
    I_dram = nc.inline_tensor(np.eye(128, dtype=np.float16), name="ident")

    mu_dram = nc.dram_tensor("mu_bounce", [2, 36, HALF], f16)

    with tile.TileContext(nc) as tc:
        with (
            tc.tile_pool(name="singles", bufs=1) as singles,
            tc.tile_pool(name="mc", bufs=3) as mc,
            tc.tile_pool(name="pd", bufs=2) as pd,
            tc.tile_pool(name="pr", bufs=2) as pr,
        ):
            # persistent SBUF
            x16 = singles.tile([C, 3, HS + 2, W], f16, tag="x16")  # dx -1,0,+1
            k1pad = singles.tile([CC, HS + 2, W + 2], f16, tag="k1pad")
            e_sb = singles.tile([KC, HS, W], f16, tag="e_sb")
            en_sb = singles.tile([KC, HS, W], f16, tag="en_sb")
            rexp16 = singles.tile([KC, HS, W], f16, tag="rexp16")
            lnS = singles.tile([4, HS, W], f16, tag="lnS")
            mu16 = singles.tile([36, HS, W], f16, tag="mu16")
            out16 = singles.tile([C, 4, HS, W], f16, tag="out16")
            cwT_sb = singles.tile([C, CC], f16, tag="cwT")
            cb_sb = singles.tile([CC, 1], f32, tag="cb")
            ewT_sb = singles.tile([CC, 9, KC], f16, tag="ewT")
            eb_sb = singles.tile([KC, 1], f32, tag="eb")
            A_sb = singles.tile([KC, 40], f16, tag="A_sb")
            A1_sb = singles.tile([KC, 12 * 128], f16, tag="A1_sb")
            R100_sb = singles.tile([4, KC], f16, tag="R100_sb")
            id_sb = singles.tile([128, 128], f16, tag="id_sb")

            nc.gpsimd.memset(x16[:, 0, :, 0:1], 0.0)
            nc.gpsimd.memset(x16[:, 2, :, W - 1 : W], 0.0)
            nc.gpsimd.memset(k1pad[:, :, 0:1], 0.0)
            nc.gpsimd.memset(k1pad[:, :, W + 1 : W + 2], 0.0)

            # x load with fp32 -> fp16 cast (SWDGE); dx-shifted copies on GPSIMD
            nc.gpsimd.dma_start(out=x16[:, 1, 0:17, :], in_=xin[:, 0:17, :])
            nc.gpsimd.dma_start(out=x16[:, 1, 17:, :], in_=xin[:, 17:, :])
            nc.sync.dma_start(out=cwT_sb, in_=cwT[:])
            nc.sync.dma_start(out=cb_sb, in_=cb[:])
            nc.sync.dma_start(out=ewT_sb, in_=ewT[:])
            nc.sync.dma_start(out=eb_sb, in_=eb[:])
            nc.sync.dma_start(out=A_sb, in_=A_dram[:])
            nc.sync.dma_start(out=A1_sb, in_=A1_dram[:])
            nc.sync.dma_start(out=R100_sb, in_=R100_dram[:])
            nc.sync.dma_start(out=id_sb, in_=I_dram[:])

            nc.vector.tensor_copy(x16[:, 0, :, 1:W], x16[:, 1, :, 0 : W - 1])
            nc.vector.tensor_copy(x16[:, 2, :, 0 : W - 1], x16[:, 1, :, 1:W])

            with (
                tc.tile_pool(name="ps_m", bufs=2, space="PSUM") as ps_m,
                tc.tile_pool(name="ps_n", bufs=1, space="PSUM") as ps_n,
            ):
                # compress conv over all 34 rows, 8-row chunks
                for ck, (r0, r1) in enumerate(
                        [(0, 8), (8, 16), (16, 24), (24, 32), (32, 34)]):
                    ps_k = ps_m.tile([CC, r1 - r0, W], f32, tag="ps_k",
                                     name=f"psk_{ck}")
                    nc.tensor.matmul(ps_k, cwT_sb, x16[:, 1, r0:r1, :],
                                     start=True, stop=True)
                    nc.scalar.activation(k1pad[:, r0:r1, 1 : 1 + W],
                                         ps_k, AF.Identity, bias=cb_sb,
                                         scale=1.0)

                # 8-row slabs: the first reassembly slot only waits for two
                # slabs instead of the whole mask phase
                bounce = [[None, None], [None, None]]
                for sl in range(4):
                    y0 = 8 * sl
                    rr = slice(y0, y0 + 8)
                    # encoder 3x3 on padded grid -> exp -> e
                    ps_e = ps_m.tile([KC, 8, W], f32, tag="ps_e",
                                     name=f"pse_{sl}")
                    for di in range(3):
                        for dj in range(3):
                            tap = di * 3 + dj
                            nc.tensor.matmul(
                                ps_e,
                                ewT_sb[:, tap, :],
                                k1pad[:, y0 + di : y0 + di + 8, dj : dj + W],
                                start=(tap == 0), stop=(tap == 8))
                    nc.scalar.activation(e_sb[:, rr, :], ps_e, AF.Exp,
                                         bias=eb_sb, scale=1.0)

                    # softmax denominators -> lnS -> r_exp (1/S per e-row)
                    ps_s = ps_n.tile([4, 8, W], f32, tag="ps_s", name=f"pss_{sl}")
                    nc.tensor.matmul(ps_s, A_sb[:, 36:40], e_sb[:, rr, :],
                                     start=True, stop=True)
                    nc.scalar.activation(lnS[:, rr, :], ps_s, AF.Ln)
                    ps_r = ps_m.tile([KC, 8, W], f32, tag="ps_e",
                                     name=f"psr_{sl}")
                    nc.tensor.matmul(ps_r, R100_sb, lnS[:, rr, :],
                                     start=True, stop=True)
                    nc.scalar.activation(rexp16[:, rr, :], ps_r,
                                         AF.Exp, scale=-1.0)
                    # normalized e
                    nc.vector.tensor_mul(en_sb[:, rr, :], e_sb[:, rr, :],
                                         rexp16[:, rr, :])

                    # normalized masks for the DMA-broadcast taps
                    ps_mu = ps_n.tile([36, 8, W], f32, tag="ps_s",
                                      name=f"psmu_{sl}")
                    nc.tensor.matmul(ps_mu, A_sb[:, 0:36], en_sb[:, rr, :],
                                     start=True, stop=True)
                    nc.scalar.activation(mu16[:, rr, :], ps_mu, AF.Copy)
                    h, sh = divmod(sl, 2)
                    bounce[h][sh] = nc.sync.dma_start(
                        out=mu_dram[h][:, 512 * sh : 512 * sh + 512],
                        in_=mu16[:, rr, :])

            # reassembly main loop (h-major: all of half 0 runs while the
            # half-1 mask phase is still in flight)
            with (
                tc.tile_pool(name="ps_acc", bufs=2, space="PSUM") as ps_acc,
                tc.tile_pool(name="ps_b", bufs=2, space="PSUM") as ps_b,
            ):
                for h in range(2):
                    for q in range(4):
                        y0 = 16 * h
                        # 6-tap broadcast (dx in {-1, 0}) from DRAM bounce
                        mcast = mc.tile([C, 6, HALF], f16, tag="mcast")
                        mflat = mcast.rearrange("p t f -> p (t f)")
                        src = bass.AP(
                            tensor=mu_dram,
                            offset=(h * 36 + q * 9) * HALF,
                            ap=[[0, C], [3 * HALF, 3], [HALF, 2], [1, HALF]])
                        bc = nc.sync.dma_start(out=mflat, in_=src)
                        for bnc in bounce[h]:
                            add_dep_helper(bc.ins, bnc.ins, sync=True,
                                           reason="mask broadcast after bounce")

                        # 3-tap PE broadcast (dx = +1) from e_norm
                        pdrain = pd.tile([C, 3, HALF], f16, tag="pdrain")
                        for k in range(3):
                            ps_bc = ps_b.tile([C, HALF], f32, tag="ps_bc",
                                              name=f"psbc_{q}_{h}_{k}")
                            a1 = A1_sb[:, (3 * q + k) * 128 : (3 * q + k + 1) * 128]
                            for hh in range(2):
                                nc.tensor.matmul(
                                    ps_bc[:, 512 * hh : 512 * hh + 512],
                                    a1,
                                    en_sb[:, y0 + 8 * hh : y0 + 8 * hh + 8, :],
                                    start=True, stop=True)
                            if q < 3:
                                nc.scalar.activation(pdrain[:, k], ps_bc, AF.Copy)
                            else:
                                nc.vector.tensor_copy(pdrain[:, k], ps_bc)

                        # products: batched fp16 TT muls
                        prod = pr.tile([C, 9, HALF], f16, tag="prod")
                        pp = 3 * (HS + 2) * W  # x16 partition pitch (elements)
                        xa = bass.AP(
                            tensor=x16.tensor, offset=x16.offset + 16 * h * W,
                            ap=[[pp, C], [W, 3], [(HS + 2) * W, 2], [1, HALF]])
                        nc.vector.tensor_mul(
                            prod[:, 0:6].rearrange("p (a b) f -> p a b f", a=3),
                            xa, mcast.rearrange("p (a b) f -> p a b f", a=3))
                        xb = bass.AP(
                            tensor=x16.tensor,
                            offset=x16.offset + 2 * (HS + 2) * W + 16 * h * W,
                            ap=[[pp, C], [W, 3], [1, HALF]])
                        nc.vector.tensor_mul(prod[:, 6:9], xb, pdrain)

                        # 9-term accumulate on PE
                        acc = ps_acc.tile([C, HALF], f32, tag="acc",
                                          name=f"acc_{q}_{h}")
                        for t in range(9):
                            for c2 in range(2):
                                nc.tensor.matmul(
                                    acc[:, 512 * c2 : 512 * c2 + 512], id_sb,
                                    prod[:, t, 512 * c2 : 512 * c2 + 512],
                                    start=(t == 0), stop=(t == 8),
                                    skip_group_check=True)
                        if q < 2:
                            nc.scalar.activation(
                                out16[:, q, y0 : y0 + 16, :], acc, AF.Copy)
                        else:
                            nc.vector.tensor_copy(
                                out16[:, q, y0 : y0 + 16, :], acc)
                        nc.sync.dma_start(out=out[:, q, y0 : y0 + 16, :],
                                          in_=out16[:, q, y0 : y0 + 16, :])

    nc.compile()
    return nc


def _get_program():
    global _PROGRAM
    if _PROGRAM is None:
        _PROGRAM = _build_program()
    return _PROGRAM


def _shard_inputs(x, comp_w, comp_b, enc_w, enc_b):
    comp_wT = np.ascontiguousarray(comp_w[:, :, 0, 0].T.astype(np.float16))
    enc_wT = np.ascontiguousarray(
        np.transpose(enc_w.reshape(KC, CC, 9), (1, 2, 0)).astype(np.float16))
    cb = np.ascontiguousarray(comp_b.astype(np.float32).reshape(CC, 1))
    eb = np.ascontiguousarray(enc_b.astype(np.float32).reshape(KC, 1))
    in_maps = []
    for core in range(NCORES):
        b, h = divmod(core, 2)
        xs = np.zeros((C, HS + 2, W), dtype=np.float32)
        lo = h * HS - 1
        s0, s1 = max(0, lo), min(H, lo + HS + 2)
        xs[:, s0 - lo : s1 - lo, :] = x[b, :, s0:s1, :]
        in_maps.append({
            "xs": np.ascontiguousarray(xs),
            "comp_wT": comp_wT,
            "comp_b": cb,
            "enc_wT": enc_wT,
            "enc_b": eb,
        })
    return in_maps


def _run(inputs, trace=False):
    from concourse.bass_utils import run_bass_kernel_spmd

    nc = _get_program()
    in_maps = _shard_inputs(**inputs)
    res = run_bass_kernel_spmd(nc, in_maps, list(range(NCORES)), trace=trace)
    out = np.empty((B, C, 2 * H, 2 * W), dtype=np.float32)
    for core in range(NCORES):
        b, h = divmod(core, 2)
        o = res.results[core]["out"].astype(np.float32)  # [C, 4, HS, W]
        blk = out[b, :, h * 2 * HS : (h + 1) * 2 * HS, :]
        for r1 in range(2):
            for r2 in range(2):
                blk[:, r1::2, r2::2] = o[:, 2 * r1 + r2]
    return out, res.exec_time_ns


def kernel(x, comp_w, comp_b, enc_w, enc_b):
    out, _ = _run(dict(x=np.asarray(x), comp_w=np.asarray(comp_w),
                       comp_b=np.asarray(comp_b), enc_w=np.asarray(enc_w),
                       enc_b=np.asarray(enc_b)))
    return out


# revision 23
# speedup vs baseline: 1.3671x; 1.0616x over previous
"""CARAFE content-aware upsampling (scale=2, K=5, encoder 3x3) on 8 TRN2 NeuronCores.

Sharding: 8 shards = batch(4) x H-halves(2), pure data parallel (1-row halo per
shard handled host-side). Channel-major fp16 pipeline, v2:

  1. compress 1x1 conv       : PE matmul, ACT drain fused with +cb bias
  2. encoder 3x3 conv        : 9 accumulating PE matmuls on padded k1
  3. e = exp(enc + eb)       : ACT
  4. S, lnS, r_exp           : PE column-sum matmul, ACT Ln, PE 4->100 expand,
                               ACT Exp(-x) drain  -> r_exp = 1/S per channel row
  5. e_norm = e * r_exp      : one DVE TT mul [100, PIX]; masks are now
                               pre-normalized at the e level
  6. mask supply (36 taps)   :  - dx in {-1,0} (24 taps): Mu rows via A-matmul,
                                  ACT drain, DRAM bounce, per-(q,half) stride-0
                                  broadcast DMA into [128, 6, 1024] tiles
                               - dx = +1 (12 taps): PE broadcast matmul
                                  (stationary = A column replicated 128x) from
                                  e_norm, drained by ACT (q0-2) / DVE (q3)
  7. products                : batched DVE TT muls (6-tap and 3-tap batches,
                               fp16 2x mode) against dx-stacked x copies
  8. accumulate              : 9-term PE identity matmuls into PSUM (fp32)
  9. out                     : ACT/DVE cast drain to fp16, contiguous per-subgrid
                               DRAM store; host does pixel-shuffle + fp32 cast
"""

import numpy as np

SCALE, KK, EK = 2, 5, 3
B, C, H, W = 4, 128, 64, 64
CC, KC = 64, 100
HS = H // 2          # 32 interior rows per shard
PIX = HS * W
HALF = PIX // 2      # 1024
NCORES = 8

_PROGRAM = None


def _build_A():
    A = np.zeros((KC, 40), dtype=np.float32)
    for r1 in range(2):
        for r2 in range(2):
            q = 2 * r1 + r2
            for i in range(KK):
                for j in range(KK):
                    dy = (r1 + i - 2) // 2
                    dx = (r2 + j - 2) // 2
                    tidx = (dy + 1) * 3 + (dx + 1)
                    A[4 * (5 * i + j) + q, q * 9 + tidx] += 1.0
            A[np.arange(q, KC, 4), 36 + q] = 1.0
    return A


def _patch_act_tables():
    # Force every activation (Exp/Ln/Identity/Copy) onto the one table set
    # that contains them all, so the kernel pays a single ACT_TABLE_LOAD
    # instead of thrashing between exp_and_others and natural_log sets.
    from concourse import bacc, hw_specs
    if getattr(bacc, "_carafe_act_patch", False):
        return
    orig = hw_specs.get_activation_tables

    def patched(arch):
        t = orig(arch)
        return {
            k: (v if k == "natural_log_exp_and_others" else set())
            for k, v in t.items()
        }

    hw_specs.get_activation_tables = patched
    bacc.get_activation_tables = patched
    bacc._carafe_act_patch = True


def _build_program():
    import concourse.bass as bass
    import concourse.tile as tile
    from concourse.tile import add_dep_helper
    from concourse import bacc, mybir

    _patch_act_tables()

    f32 = mybir.dt.float32
    f16 = mybir.dt.float16
    AF = mybir.ActivationFunctionType

    nc = bacc.Bacc("TRN2", target_bir_lowering=False, debug=False,
                   num_devices=NCORES)

    xin = nc.declare_dram_parameter("xs", [C, HS + 2, W], f32, isOutput=False)
    cwT = nc.declare_dram_parameter("comp_wT", [C, CC], f16, isOutput=False)
    cb = nc.declare_dram_parameter("comp_b", [CC, 1], f32, isOutput=False)
    ewT = nc.declare_dram_parameter("enc_wT", [CC, 9, KC], f16, isOutput=False)
    eb = nc.declare_dram_parameter("enc_b", [KC, 1], f32, isOutput=False)
    out = nc.declare_dram_parameter("out", [C, 4, HS, W], f16, isOutput=True)

    A_np = _build_A().astype(np.float16)
    A_dram = nc.inline_tensor(A_np, name="A_cmb")
    # PE-broadcast stationaries: for each (q, dy) the dx=+1 mask column of A,
    # replicated across 128 output partitions.
    A1 = np.zeros((KC, 12, 128), dtype=np.float16)
    for q in range(4):
        for k in range(3):          # k = dy + 1
            tidx = 3 * k + 2        # dx = +1
            A1[:, 3 * q + k, :] = A_np[:, 9 * q + tidx][:, None]
    A1_dram = nc.inline_tensor(A1.reshape(KC, 12 * 128), name="A1_bcast")
    R100 = np.zeros((4, KC), dtype=np.float16)
    for j in range(KC):
        R100[j % 4, j] = 1.0
    R100_dram = nc.inline_tensor(R100, name="R100")
    I_dram = nc.inline_tensor(np.eye(128, dtype=np.float16), name="ident")

    mu_dram = nc.dram_tensor("mu_bounce", [2, 36, HALF], f16)

    with tile.TileContext(nc) as tc:
        with (
            tc.tile_pool(name="singles", bufs=1) as singles,
            tc.tile_pool(name="mc", bufs=3) as mc,
            tc.tile_pool(name="pd", bufs=2) as pd,
            tc.tile_pool(name="pr", bufs=2) as pr,
        ):
            # persistent SBUF
            x16 = singles.tile([C, 3, HS + 2, W], f16, tag="x16")  # dx -1,0,+1
            k1pad = singles.tile([CC, HS + 2, W + 2], f16, tag="k1pad")
            e_sb = singles.tile([KC, HS, W], f16, tag="e_sb")
            en_sb = singles.tile([KC, HS, W], f16, tag="en_sb")
            rexp16 = singles.tile([KC, HS, W], f16, tag="rexp16")
            lnS = singles.tile([4, HS, W], f16, tag="lnS")
            mu16 = singles.tile([36, HS, W], f16, tag="mu16")
            out16 = singles.tile([C, 4, HS, W], f16, tag="out16")
            cwT_sb = singles.tile([C, CC], f16, tag="cwT")
            cb_sb = singles.tile([CC, 1], f32, tag="cb")
            ewT_sb = singles.tile([CC, 9, KC], f16, tag="ewT")
            eb_sb = singles.tile([KC, 1], f32, tag="eb")
            A_sb = singles.tile([KC, 40], f16, tag="A_sb")
            A1_sb = singles.tile([KC, 12 * 128], f16, tag="A1_sb")
            R100_sb = singles.tile([4, KC], f16, tag="R100_sb")
            id_sb = singles.tile([128, 128], f16, tag="id_sb")

            nc.gpsimd.memset(x16[:, 0, :, 0:1], 0.0)
            nc.gpsimd.memset(x16[:, 2, :, W - 1 : W], 0.0)
            nc.gpsimd.memset(k1pad[:, :, 0:1], 0.0)
            nc.gpsimd.memset(k1pad[:, :, W + 1 : W + 2], 0.0)

            # x load with fp32 -> fp16 cast (SWDGE); dx-shifted copies on GPSIMD
            nc.gpsimd.dma_start(out=x16[:, 1, 0:17, :], in_=xin[:, 0:17, :])
            nc.gpsimd.dma_start(out=x16[:, 1, 17:, :], in_=xin[:, 17:, :])
            nc.sync.dma_start(out=cwT_sb, in_=cwT[:])
            nc.sync.dma_start(out=cb_sb, in_=cb[:])
            nc.sync.dma_start(out=ewT_sb, in_=ewT[:])
            nc.sync.dma_start(out=eb_sb, in_=eb[:])
            nc.sync.dma_start(out=A_sb, in_=A_dram[:])
            nc.sync.dma_start(out=A1_sb, in_=A1_dram[:])
            nc.sync.dma_start(out=R100_sb, in_=R100_dram[:])
            nc.sync.dma_start(out=id_sb, in_=I_dram[:])

            nc.vector.tensor_copy(x16[:, 0, :, 1:W], x16[:, 1, :, 0 : W - 1])
            nc.vector.tensor_copy(x16[:, 2, :, 0 : W - 1], x16[:, 1, :, 1:W])

            with (
                tc.tile_pool(name="ps_m", bufs=3, space="PSUM") as ps_m,
                tc.tile_pool(name="ps_k2", bufs=2, space="PSUM") as ps_k2,
                tc.tile_pool(name="ps_n", bufs=1, space="PSUM") as ps_n,
            ):
                # compress conv over all 34 rows, 8-row chunks
                for ck, (r0, r1) in enumerate(
                        [(0, 8), (8, 16), (16, 24), (24, 32), (32, 34)]):
                    ps_k = ps_k2.tile([CC, r1 - r0, W], f32, tag="ps_k",
                                      name=f"psk_{ck}")
                    nc.tensor.matmul(ps_k, cwT_sb, x16[:, 1, r0:r1, :],
                                     start=True, stop=True)
                    nc.scalar.activation(k1pad[:, r0:r1, 1 : 1 + W],
                                         ps_k, AF.Identity, bias=cb_sb,
                                         scale=1.0)

                # encoder slabs (dense PE streams) + per-half normalization
                # chains, ordered so slab s+1's encoder is never queued behind
                # a matmul that stalls on the ACT/DVE chain of slab s.
                bounce = [None, None]

                def encoder_slab(sl):
                    y0 = 8 * sl
                    ps_e = ps_m.tile([KC, 8, W], f32, tag="ps_e",
                                     name=f"pse_{sl}")
                    for di in range(3):
                        for dj in range(3):
                            tap = di * 3 + dj
                            nc.tensor.matmul(
                                ps_e,
                                ewT_sb[:, tap, :],
                                k1pad[:, y0 + di : y0 + di + 8, dj : dj + W],
                                start=(tap == 0), stop=(tap == 8))
                    nc.scalar.activation(e_sb[:, y0 : y0 + 8, :], ps_e, AF.Exp,
                                         bias=eb_sb, scale=1.0)

                def norm_half(h):
                    y0 = 16 * h
                    rr = slice(y0, y0 + 16)
                    ps_s = ps_n.tile([4, 16, W], f32, tag="ps_s",
                                     name=f"pss_{h}")
                    for hh in range(2):
                        nc.tensor.matmul(
                            ps_s[:, 8 * hh : 8 * hh + 8, :], A_sb[:, 36:40],
                            e_sb[:, y0 + 8 * hh : y0 + 8 * hh + 8, :],
                            start=True, stop=True)
                    nc.scalar.activation(lnS[:, rr, :], ps_s, AF.Ln)
                    for hh in range(2):
                        r8 = slice(y0 + 8 * hh, y0 + 8 * hh + 8)
                        ps_r = ps_m.tile([KC, 8, W], f32, tag="ps_e",
                                         name=f"psr_{h}_{hh}")
                        nc.tensor.matmul(ps_r, R100_sb, lnS[:, r8, :],
                                         start=True, stop=True)
                        nc.scalar.activation(rexp16[:, r8, :], ps_r,
                                             AF.Exp, scale=-1.0)
                        nc.vector.tensor_mul(en_sb[:, r8, :], e_sb[:, r8, :],
                                             rexp16[:, r8, :])
                    ps_mu = ps_n.tile([36, 16, W], f32, tag="ps_s",
                                      name=f"psmu_{h}")
                    for hh in range(2):
                        nc.tensor.matmul(
                            ps_mu[:, 8 * hh : 8 * hh + 8, :], A_sb[:, 0:36],
                            en_sb[:, y0 + 8 * hh : y0 + 8 * hh + 8, :],
                            start=True, stop=True)
                    nc.scalar.activation(mu16[:, rr, :], ps_mu, AF.Copy)
                    bounce[h] = nc.sync.dma_start(out=mu_dram[h],
                                                  in_=mu16[:, rr, :])

                encoder_slab(0)
                encoder_slab(1)
                norm_half(0)
                encoder_slab(2)
                encoder_slab(3)
                norm_half(1)

            # reassembly main loop (h-major: all of half 0 runs while the
            # half-1 mask phase is still in flight)
            with (
                tc.tile_pool(name="ps_acc", bufs=2, space="PSUM") as ps_acc,
                tc.tile_pool(name="ps_b", bufs=2, space="PSUM") as ps_b,
            ):
                for h in range(2):
                    for q in range(4):
                        y0 = 16 * h
                        # 6-tap broadcast (dx in {-1, 0}) from DRAM bounce
                        mcast = mc.tile([C, 6, HALF], f16, tag="mcast")
                        mflat = mcast.rearrange("p t f -> p (t f)")
                        src = bass.AP(
                            tensor=mu_dram,
                            offset=(h * 36 + q * 9) * HALF,
                            ap=[[0, C], [3 * HALF, 3], [HALF, 2], [1, HALF]])
                        bc = nc.sync.dma_start(out=mflat, in_=src)
                        add_dep_helper(bc.ins, bounce[h].ins, sync=True,
                                       reason="mask broadcast after bounce")

                        # 3-tap PE broadcast (dx = +1) from e_norm
                        pdrain = pd.tile([C, 3, HALF], f16, tag="pdrain")
                        for k in range(3):
                            ps_bc = ps_b.tile([C, HALF], f32, tag="ps_bc",
                                              name=f"psbc_{q}_{h}_{k}")
                            a1 = A1_sb[:, (3 * q + k) * 128 : (3 * q + k + 1) * 128]
                            for hh in range(2):
                                nc.tensor.matmul(
                                    ps_bc[:, 512 * hh : 512 * hh + 512],
                                    a1,
                                    en_sb[:, y0 + 8 * hh : y0 + 8 * hh + 8, :],
                                    start=True, stop=True)
                            if q < 3:
                                nc.scalar.activation(pdrain[:, k], ps_bc, AF.Copy)
                            else:
                                nc.vector.tensor_copy(pdrain[:, k], ps_bc)

                        # products: batched fp16 TT muls
                        prod = pr.tile([C, 9, HALF], f16, tag="prod")
                        pp = 3 * (HS + 2) * W  # x16 partition pitch (elements)
                        xa = bass.AP(
                            tensor=x16.tensor, offset=x16.offset + 16 * h * W,
                            ap=[[pp, C], [W, 3], [(HS + 2) * W, 2], [1, HALF]])
                        nc.vector.tensor_mul(
                            prod[:, 0:6].rearrange("p (a b) f -> p a b f", a=3),
                            xa, mcast.rearrange("p (a b) f -> p a b f", a=3))
                        xb = bass.AP(
                            tensor=x16.tensor,
                            offset=x16.offset + 2 * (HS + 2) * W + 16 * h * W,
                            ap=[[pp, C], [W, 3], [1, HALF]])
                        nc.vector.tensor_mul(prod[:, 6:9], xb, pdrain)

                        # 9-term accumulate on PE
                        acc = ps_acc.tile([C, HALF], f32, tag="acc",
                                          name=f"acc_{q}_{h}")
                        for t in range(9):
                            for c2 in range(2):
                                nc.tensor.matmul(
                                    acc[:, 512 * c2 : 512 * c2 + 512], id_sb,
                                    prod[:, t, 512 * c2 : 512 * c2 + 512],
                                    start=(t == 0), stop=(t == 8),
                                    skip_group_check=True)
                        nc.scalar.activation(
                            out16[:, q, y0 : y0 + 16, :], acc, AF.Copy)
                        nc.sync.dma_start(out=out[:, q, y0 : y0 + 16, :],
                                          in_=out16[:, q, y0 : y0 + 16, :])

    nc.compile()
    return nc


def _get_program():
    global _PROGRAM
    if _PROGRAM is None:
        _PROGRAM = _build_program()
    return _PROGRAM


def _shard_inputs(x, comp_w, comp_b, enc_w, enc_b):
    comp_wT = np.ascontiguousarray(comp_w[:, :, 0, 0].T.astype(np.float16))
    enc_wT = np.ascontiguousarray(
        np.transpose(enc_w.reshape(KC, CC, 9), (1, 2, 0)).astype(np.float16))
    cb = np.ascontiguousarray(comp_b.astype(np.float32).reshape(CC, 1))
    eb = np.ascontiguousarray(enc_b.astype(np.float32).reshape(KC, 1))
    in_maps = []
    for core in range(NCORES):
        b, h = divmod(core, 2)
        xs = np.zeros((C, HS + 2, W), dtype=np.float32)
        lo = h * HS - 1
        s0, s1 = max(0, lo), min(H, lo + HS + 2)
        xs[:, s0 - lo : s1 - lo, :] = x[b, :, s0:s1, :]
        in_maps.append({
            "xs": np.ascontiguousarray(xs),
            "comp_wT": comp_wT,
            "comp_b": cb,
            "enc_wT": enc_wT,
            "enc_b": eb,
        })
    return in_maps


def _run(inputs, trace=False):
    from concourse.bass_utils import run_bass_kernel_spmd

    nc = _get_program()
    in_maps = _shard_inputs(**inputs)
    res = run_bass_kernel_spmd(nc, in_maps, list(range(NCORES)), trace=trace)
    out = np.empty((B, C, 2 * H, 2 * W), dtype=np.float32)
    for core in range(NCORES):
        b, h = divmod(core, 2)
        o = res.results[core]["out"].astype(np.float32)  # [C, 4, HS, W]
        blk = out[b, :, h * 2 * HS : (h + 1) * 2 * HS, :]
        for r1 in range(2):
            for r2 in range(2):
                blk[:, r1::2, r2::2] = o[:, 2 * r1 + r2]
    return out, res.exec_time_ns


def kernel(x, comp_w, comp_b, enc_w, enc_b):
    out, _ = _run(dict(x=np.asarray(x), comp_w=np.asarray(comp_w),
                       comp_b=np.asarray(comp_b), enc_w=np.asarray(enc_w),
                       enc_b=np.asarray(enc_b)))
    return out
